# revision 12
# baseline (speedup 1.0000x reference)
"""ComplexLSTM Trainium2 kernel.

Problem: B=32, I=128, H=256, T=2048. Four independent LSTM scans
(real/imag weights x real/imag inputs) combined into a complex output
(B, H, T) complex64.

Sharding: data-parallel over batch across 8 cores (4 rows each); each
core runs all four scans for its batch slice, organized as two "chains"
that share a recurrent weight matrix (Whh_r / Whh_i).

Phase-2 step layout (per chain, rows R=8 = 2 slots x 4 batch):
  gates PSUM tile [40, 512], one bank: rows 0:8 = gates [i|f],
  rows 32:40 = gates [g|o] (matmul col-tile bases must be 0/32/64).
  gx(+bias) is preloaded into the bank by a scatter-identity matmul
  (K=16 -> M=40), then 4 accumulating matmuls (2 h-chunks x 2 slices)
  add h @ Whh.T. One sigmoid covers all gates (g rows pre-scaled by 2
  on host: tanh(x) = 2*sigmoid(2x)-1).
  c' = Sf*c + 2*Si*Sg - Si (GPSIMD products, DVE combine)
  h  = So * tanh(c')        (ACT + DVE)
  h is transposed on the PE (4 small transposes -> one [128,32] PSUM
  tile) and copied to SBUF with a single ACT copy as next step's
  stationary. The scatter for step k+1 is issued before step k's
  transposes so the PE has work while the elementwise tail drains.
  The real/imag combine is deferred to phase 3.
"""

import numpy as np
from contextlib import ExitStack

import concourse.bass as bass
import concourse.tile as tile
import concourse.mybir as mybir
from concourse import bacc
from concourse.bass import ds
from concourse.bass_utils import run_bass_kernel_spmd
from concourse.masks import make_identity

B, I, H = 32, 128, 256
G = 4 * H            # 1024
NCORES = 8
BL = B // NCORES     # 4 batch rows per core
R = 2 * BL           # 8 rows per chain (2 slots x 4 batch)
U = 16               # steps per For_i iteration

f32 = mybir.dt.float32
f32r = mybir.dt.float32r
bf16 = mybir.dt.bfloat16
SIG = mybir.ActivationFunctionType.Sigmoid
TANH = mybir.ActivationFunctionType.Tanh
MULT = mybir.AluOpType.mult
SUB = mybir.AluOpType.subtract


def build_program(T):
    import os
    skip1 = os.environ.get("K_SKIP1") == "1"
    skip2 = os.environ.get("K_SKIP2") == "1"
    skip3 = os.environ.get("K_SKIP3") == "1"
    TC = T // 128      # phase-1/3 tiles per (scan, b)
    nc = bacc.Bacc("TRN2", target_bir_lowering=False, debug=False,
                   num_devices=NCORES)

    xr = nc.declare_dram_parameter("xr", [BL, I, T], f32r, isOutput=False)
    xi = nc.declare_dram_parameter("xi", [BL, I, T], f32r, isOutput=False)
    wih = nc.declare_dram_parameter("wih", [2, I, G], f32r, isOutput=False)
    whh = nc.declare_dram_parameter("whh", [2, H, G], bf16, isOutput=False)
    bia = nc.declare_dram_parameter("bias", [2, G], f32, isOutput=False)
    scat = nc.declare_dram_parameter("scat", [2 * R, 40], f32r, isOutput=False)
    out = nc.declare_dram_parameter("out", [BL, H, 2 * T], f32, isOutput=True)

    # staging: gx per chain [T, 16, 512] (rows 0:8 = [i|f], 8:16 = [g|o]);
    # raw h per chain [T, R, H]
    gxst = [nc.dram_tensor(f"gx_stage{c}", [T, 2 * R, 512], f32r)
            for c in range(2)]
    hst = [nc.dram_tensor(f"h_stage{c}", [T, R, H], bf16) for c in range(2)]

    # chain c, slot s -> input tensor (chain0=Wr: xr,xi ; chain1=Wi: xi,xr)
    def xsrc(c, s):
        return (xr if s == 0 else xi) if c == 0 else (xi if s == 0 else xr)

    with tile.TileContext(nc) as tc, ExitStack() as top:
        consts = top.enter_context(tc.tile_pool(name="consts", bufs=1))

        whh_sb = [[consts.tile([128, G], bf16, name=f"whh{c}{k}",
                               tag=f"whh{c}{k}") for k in range(2)]
                  for c in range(2)]
        for c in range(2):
            for k in range(2):
                nc.sync.dma_start(out=whh_sb[c][k],
                                  in_=whh[c, k * 128:(k + 1) * 128, :])

        scat_sb = consts.tile([2 * R, 40], f32r, tag="scat_sb")
        nc.sync.dma_start(out=scat_sb, in_=scat[:, :])
        id8b = consts.tile([R, R], bf16, tag="id8b")
        make_identity(nc, id8b)

        # ---------------- phase 1: gx = x @ WihT + bias ----------------
        with ExitStack() as p1:
          if not skip1:
            p1c = p1.enter_context(tc.tile_pool(name="p1c", bufs=1))
            xp = p1.enter_context(tc.tile_pool(name="xp", bufs=4))
            gp = p1.enter_context(tc.tile_pool(name="gp", bufs=2, space="PSUM"))
            gs = p1.enter_context(tc.tile_pool(name="gs", bufs=4))

            wih_sb = [p1c.tile([I, G], f32r, name=f"wih{c}", tag=f"wih{c}")
                      for c in range(2)]
            bia_sb = [p1c.tile([128, G], f32, name=f"bia{c}", tag=f"bia{c}")
                      for c in range(2)]
            for c in range(2):
                nc.sync.dma_start(out=wih_sb[c], in_=wih[c])
                bsrc = bia[c:c + 1, :]
                nc.sync.dma_start(
                    out=bia_sb[c],
                    in_=bass.AP(tensor=bsrc.tensor, offset=bsrc.offset,
                                ap=[[0, 128]] + list(bsrc.ap[-1:])))

            for c in range(2):
                for s in range(2):
                    src = xsrc(c, s)
                    for b in range(BL):
                        row = s * BL + b
                        for t in range(TC):
                            xt = xp.tile([I, 128], f32r, tag="xt")
                            nc.sync.dma_start(
                                out=xt, in_=src[b, :, t * 128:(t + 1) * 128])
                            ps = gp.tile([128, G], f32, tag="ps")
                            for n in range(2):
                                sl = ds(n * 512, 512)
                                nc.tensor.matmul(ps[:, sl], xt,
                                                 wih_sb[c][:, sl],
                                                 start=True, stop=True)
                            gt = gs.tile([128, G], f32r, tag="gt")
                            nc.vector.tensor_add(gt, ps, bia_sb[c])
                            tsl = ds(t * 128, 128)
                            nc.sync.dma_start(
                                out=gxst[c][tsl, row, :], in_=gt[:, 0:512])
                            nc.sync.dma_start(
                                out=gxst[c][tsl, R + row, :],
                                in_=gt[:, 512:1024])

        # ---------------- phase 2: the recurrence ----------------
        with ExitStack() as p2:
          if not skip2:
            st8 = p2.enter_context(tc.tile_pool(name="st8", bufs=1))
            gxp = p2.enter_context(tc.tile_pool(name="gxp", bufs=2))
            spl = p2.enter_context(tc.tile_pool(name="spl", bufs=2))
            tmp = p2.enter_context(tc.tile_pool(name="tmp", bufs=2))
            stg = p2.enter_context(tc.tile_pool(name="stg", bufs=2))
            psA = p2.enter_context(tc.tile_pool(name="psA", bufs=3, space="PSUM"))
            psB = p2.enter_context(tc.tile_pool(name="psB", bufs=3, space="PSUM"))
            psT = p2.enter_context(tc.tile_pool(name="psT", bufs=1, space="PSUM"))

            # persistent state (ping-pong on step parity), split per chain
            # so the two chains' dependency cycles stay independent.
            # hTc[c][p]: [128, 16] cols hb*8..+8 = h[c]^T chunk hb
            # c state lives at base partition 32 (rows 32:40) to match the
            # [f|o] gate zone
            hTc = [[st8.tile([128, 2 * R], bf16, name=f"hT{c}{p}",
                             tag=f"hT{c}{p}") for p in range(2)]
                   for c in range(2)]
            cst = [[st8.tile([40, H], f32, name=f"c{c}{p}", tag=f"c{c}{p}")
                    for p in range(2)] for c in range(2)]
            for p in range(2):
                for c in range(2):
                    nc.vector.memset(hTc[c][p], 0.0)
                    nc.vector.memset(cst[c][p], 0.0)

            stt = nc.vector.scalar_tensor_tensor

            def hT_sl(p, c, hb):
                return hTc[c][p][:, hb * R:(hb + 1) * R]

            with tc.For_i(0, T, U, staggered_reset=True) as iv:
                gxch = [gxp.tile([2 * R, U, 512], f32r, name=f"gx{c}",
                                 tag=f"gx{c}") for c in range(2)]
                for c in range(2):
                    # split the load so the first steps' gx lands quickly
                    # after the loop barrier and the k=0 scatter can start
                    nc.sync.dma_start(
                        out=gxch[c][:, 0:2, :],
                        in_=gxst[c][ds(iv, 2), :, :].rearrange(
                            "u p g -> p u g"))
                    nc.sync.dma_start(
                        out=gxch[c][:, 2:U, :],
                        in_=gxst[c][ds(iv + 2, U - 2), :, :].rearrange(
                            "u p g -> p u g"))
                st = [stg.tile([R, U, H], bf16, name=f"st{c}", tag=f"st{c}")
                      for c in range(2)]

                def scatter(c, kk):
                    pool = psA if c == 0 else psB
                    g_ = pool.tile([40, 512], f32, name=f"G{c}", tag=f"G{c}")
                    nc.tensor.matmul(g_, scat_sb, gxch[c][:, kk, :],
                                     start=True, stop=True)
                    return g_

                Gq = []
                for k in range(U):
                    pp = k % 2
                    if k == 0:
                        Gq = [[scatter(c, 0) for c in range(2)],
                              [scatter(c, 1) for c in range(2)]]
                    Gcur = Gq.pop(0)
                    # prefetch gx two steps ahead into fresh psum banks:
                    # the PE runs these while the gate matmuls below still
                    # wait on the recurrent state
                    if k + 2 < U:
                        Gq.append([scatter(c, k + 2) for c in range(2)])
                    # gate matmuls accumulate onto gx (h-chunk outer so
                    # the first two only wait on hT chunk 0)
                    for c in range(2):
                        for kc in range(2):
                            for s in range(2):
                                nc.tensor.matmul(
                                    Gcur[c][s * 32:s * 32 + R, :],
                                    hT_sl(pp, c, kc),
                                    whh_sb[c][kc][:, s * 512:(s + 1) * 512],
                                    start=False, stop=True,
                                    skip_group_check=True)
                    # sigmoids (gate cols permuted on host to [i g | f o]:
                    # zone0 rows 0:8 = i|g, zone32 rows 32:40 = f|o)
                    S, slc = [], {}
                    for c in range(2):
                        S_ = spl.tile([40, 512], bf16, name=f"S{c}",
                                      tag=f"S{c}")
                        nc.scalar.activation(S_, Gcur[c], SIG)
                        S.append(S_)
                        slc[c] = (S_[0:R, 0:256], S_[32:32 + R, 0:256],
                                  S_[0:R, 256:512], S_[32:32 + R, 256:512])
                    # c update: GPSIMD takes v = Sf*c (off the critical
                    # cycle); DVE runs each chain's dependent tail
                    # [p, u, cn, h] consecutively so chain A's tail is not
                    # queued behind chain B's ops
                    for c in range(2):
                        Sf = slc[c][1]
                        v_ = tmp.tile([R, H], f32, name=f"v{c}", tag=f"v{c}")
                        nc.gpsimd.tensor_mul(v_, Sf, cst[c][pp][32:40, :])
                        slc[c] = slc[c] + (v_,)
                    for c in range(2):
                        Si, Sf, Sg, So, v_ = slc[c]
                        p_ = tmp.tile([R, H], bf16, name=f"p{c}", tag=f"p{c}")
                        nc.vector.tensor_mul(p_, Si, Sg)
                        u_ = tmp.tile([R, H], bf16, name=f"u{c}", tag=f"u{c}")
                        stt(out=u_, in0=p_, scalar=2.0, in1=Si,
                            op0=MULT, op1=SUB)
                        cn = cst[c][1 - pp][32:40, :]
                        nc.vector.tensor_add(cn, u_, v_)
                        tc_ = tmp.tile([40, H], bf16, name=f"tc{c}",
                                       tag=f"tc{c}")
                        nc.scalar.activation(tc_[32:40, :], cn, TANH)
                        # h = sig(o) * tanh(c) -> store buffer (bf16)
                        nc.vector.tensor_mul(st[c][:, k, :], So,
                                             tc_[32:40, :])
                        ptt = psT.tile([128, 2 * R], bf16, name=f"ptt{c}",
                                       tag=f"ptt{c}")
                        # per-chunk transpose+copy: the next step's hb=0
                        # matmuls only wait for chunk 0
                        nc.tensor.transpose(ptt[:, 0:R],
                                            st[c][:, k, 0:128], id8b)
                        nc.scalar.copy(hTc[c][1 - pp][:, 0:R], ptt[:, 0:R])
                        nc.tensor.transpose(ptt[:, R:2 * R],
                                            st[c][:, k, 128:256], id8b)
                        nc.vector.tensor_copy(hTc[c][1 - pp][:, R:2 * R],
                                              ptt[:, R:2 * R])
                for c in range(2):
                    nc.gpsimd.dma_start(
                        out=hst[c][ds(iv, U), :, :].rearrange("u p h -> p u h"),
                        in_=st[c])

        # ------- phase 3: combine, transpose to (b, h, t), interleave -----
        with ExitStack() as p3:
          if not skip3:
            p3c = p3.enter_context(tc.tile_pool(name="p3c", bufs=1))
            lp = p3.enter_context(tc.tile_pool(name="lp", bufs=4))
            cmb = p3.enter_context(tc.tile_pool(name="cmb", bufs=4))
            tp = p3.enter_context(tc.tile_pool(name="tp", bufs=4, space="PSUM"))
            op = p3.enter_context(tc.tile_pool(name="op", bufs=4))

            id128 = p3c.tile([128, 128], f32, tag="id128")
            make_identity(nc, id128)

            for b in range(BL):
                for t in range(TC):
                    tsl = ds(t * 128, 128)
                    a0 = lp.tile([128, H], bf16, tag="a0")
                    b0 = lp.tile([128, H], bf16, tag="b0")
                    a1 = lp.tile([128, H], bf16, tag="a1")
                    b1 = lp.tile([128, H], bf16, tag="b1")
                    nc.sync.dma_start(out=a0, in_=hst[0][tsl, b, :])
                    nc.sync.dma_start(out=b0, in_=hst[1][tsl, b, :])
                    nc.sync.dma_start(out=a1, in_=hst[0][tsl, BL + b, :])
                    nc.sync.dma_start(out=b1, in_=hst[1][tsl, BL + b, :])
                    lr = cmb.tile([128, H], f32, tag="lr")
                    nc.vector.tensor_sub(lr, a0, b0)
                    li = cmb.tile([128, H], f32, tag="li")
                    nc.gpsimd.tensor_add(li, a1, b1)
                    for hb in range(2):
                        hsl = ds(hb * 128, 128)
                        ptr = tp.tile([128, 128], f32, tag="ptr")
                        nc.tensor.transpose(ptr, lr[:, hsl], id128)
                        pti = tp.tile([128, 128], f32, tag="pti")
                        nc.tensor.transpose(pti, li[:, hsl], id128)
                        ot = op.tile([128, 256], f32, tag="ot")
                        otv = ot.rearrange("p (t two) -> p t two", two=2)
                        nc.vector.tensor_copy(otv[:, :, 0], ptr)
                        nc.vector.tensor_copy(otv[:, :, 1], pti)
                        nc.sync.dma_start(
                            out=out[b, hsl, ds(2 * t * 128, 256)], in_=ot)

    nc.compile()
    return nc


_CACHE = {}
LAST_RES = None


def get_program(T):
    if T not in _CACHE:
        _CACHE[T] = build_program(T)
    return _CACHE[T]


def _pack_weights(Wih, Whh, bih, bhh):
    Wih = np.array(Wih, dtype=np.float32, copy=True)
    Whh = np.array(Whh, dtype=np.float32, copy=True)
    b = (np.asarray(bih) + np.asarray(bhh)).astype(np.float32)
    # pre-scale g gate (rows 2H:3H) by 2 so sigmoid(2g) gives tanh via 2s-1
    Wih[2 * H:3 * H] *= 2.0
    Whh[2 * H:3 * H] *= 2.0
    b[2 * H:3 * H] *= 2.0
    # permute gate blocks (i, f, g, o) -> (i, g, f, o) so the kernel's
    # zone0 = [i|g], zone32 = [f|o]
    perm = np.r_[0:H, 2 * H:3 * H, H:2 * H, 3 * H:4 * H]
    Wih = Wih[perm]
    Whh = Whh[perm]
    b = b[perm]
    return np.ascontiguousarray(Wih.T), np.ascontiguousarray(Whh.T), b


def kernel(x_real, x_imag, Wih_r, Whh_r, bih_r, bhh_r,
           Wih_i, Whh_i, bih_i, bhh_i):
    x_real = np.asarray(x_real, dtype=np.float32)
    x_imag = np.asarray(x_imag, dtype=np.float32)
    T = x_real.shape[2]
    nc = get_program(T)

    wihT_r, whhT_r, b_r = _pack_weights(Wih_r, Whh_r, bih_r, bhh_r)
    wihT_i, whhT_i, b_i = _pack_weights(Wih_i, Whh_i, bih_i, bhh_i)
    wih_p = np.ascontiguousarray(np.stack([wihT_r, wihT_i]))
    import ml_dtypes
    whh_p = np.ascontiguousarray(
        np.stack([whhT_r, whhT_i]).astype(ml_dtypes.bfloat16))
    bia_p = np.ascontiguousarray(np.stack([b_r, b_i]))
    scat_p = np.zeros((2 * R, 40), dtype=np.float32)
    for j in range(R):
        scat_p[j, j] = 1.0
        scat_p[R + j, 32 + j] = 1.0

    in_maps = []
    for c in range(NCORES):
        sl = slice(c * BL, (c + 1) * BL)
        in_maps.append({
            "xr": np.ascontiguousarray(x_real[sl]),
            "xi": np.ascontiguousarray(x_imag[sl]),
            "wih": wih_p, "whh": whh_p, "bias": bia_p,
            "scat": scat_p,
        })
    import os
    trace = os.environ.get("K_TRACE") == "1"
    res = run_bass_kernel_spmd(nc, in_maps, list(range(NCORES)), trace=trace)
    global LAST_RES
    LAST_RES = res
    parts = []
    for c in range(NCORES):
        o = np.ascontiguousarray(res.results[c]["out"])  # [BL, H, 2T] f32
        parts.append(o.view(np.complex64))               # [BL, H, T]
    return np.concatenate(parts, axis=0)


# revision 13
# speedup vs baseline: 1.0846x; 1.0846x over previous
"""ComplexLSTM Trainium2 kernel.

Problem: B=32, I=128, H=256, T=2048. Four independent LSTM scans
(real/imag weights x real/imag inputs) combined into a complex output
(B, H, T) complex64.

Sharding: data-parallel over batch across 8 cores (4 rows each); each
core runs all four scans for its batch slice, organized as two "chains"
that share a recurrent weight matrix (Whh_r / Whh_i).

Phase-2 step layout (per chain, rows R=8 = 2 slots x 4 batch):
  gates PSUM tile [40, 512], one bank: rows 0:8 = gates [i|f],
  rows 32:40 = gates [g|o] (matmul col-tile bases must be 0/32/64).
  gx(+bias) is preloaded into the bank by a scatter-identity matmul
  (K=16 -> M=40), then 4 accumulating matmuls (2 h-chunks x 2 slices)
  add h @ Whh.T. One sigmoid covers all gates (g rows pre-scaled by 2
  on host: tanh(x) = 2*sigmoid(2x)-1).
  c' = Sf*c + 2*Si*Sg - Si (GPSIMD products, DVE combine)
  h  = So * tanh(c')        (ACT + DVE)
  h is transposed on the PE (4 small transposes -> one [128,32] PSUM
  tile) and copied to SBUF with a single ACT copy as next step's
  stationary. The scatter for step k+1 is issued before step k's
  transposes so the PE has work while the elementwise tail drains.
  The real/imag combine is deferred to phase 3.
"""

import numpy as np
from contextlib import ExitStack

import concourse.bass as bass
import concourse.tile as tile
import concourse.mybir as mybir
from concourse import bacc
from concourse.bass import ds
from concourse.bass_utils import run_bass_kernel_spmd
from concourse.masks import make_identity

B, I, H = 32, 128, 256
G = 4 * H            # 1024
NCORES = 8
BL = B // NCORES     # 4 batch rows per core
R = 2 * BL           # 8 rows per chain (2 slots x 4 batch)
U = 16               # steps per For_i iteration

f32 = mybir.dt.float32
f32r = mybir.dt.float32r
bf16 = mybir.dt.bfloat16
SIG = mybir.ActivationFunctionType.Sigmoid
TANH = mybir.ActivationFunctionType.Tanh
MULT = mybir.AluOpType.mult
SUB = mybir.AluOpType.subtract


def build_program(T):
    import os
    skip1 = os.environ.get("K_SKIP1") == "1"
    skip2 = os.environ.get("K_SKIP2") == "1"
    skip3 = os.environ.get("K_SKIP3") == "1"
    TC = T // 128      # phase-1/3 tiles per (scan, b)
    nc = bacc.Bacc("TRN2", target_bir_lowering=False, debug=False,
                   num_devices=NCORES)

    xr = nc.declare_dram_parameter("xr", [BL, I, T], f32r, isOutput=False)
    xi = nc.declare_dram_parameter("xi", [BL, I, T], f32r, isOutput=False)
    wih = nc.declare_dram_parameter("wih", [2, I, G], f32r, isOutput=False)
    whh = nc.declare_dram_parameter("whh", [2, H, G], bf16, isOutput=False)
    bia = nc.declare_dram_parameter("bias", [2, G], f32, isOutput=False)
    scat = nc.declare_dram_parameter("scat", [2 * R, 40], f32r, isOutput=False)
    out = nc.declare_dram_parameter("out", [BL, H, 2 * T], f32, isOutput=True)

    # staging: gx per chain [T, 16, 512] (rows 0:8 = [i|f], 8:16 = [g|o]);
    # raw h per chain [T, R, H]
    # padded by 4 step-rows: the in-loop head prefetch reads iv+U..iv+U+4
    gxst = [nc.dram_tensor(f"gx_stage{c}", [T + 4, 2 * R, 512], f32r)
            for c in range(2)]
    hst = [nc.dram_tensor(f"h_stage{c}", [T, R, H], bf16) for c in range(2)]

    # chain c, slot s -> input tensor (chain0=Wr: xr,xi ; chain1=Wi: xi,xr)
    def xsrc(c, s):
        return (xr if s == 0 else xi) if c == 0 else (xi if s == 0 else xr)

    with tile.TileContext(nc) as tc, ExitStack() as top:
        consts = top.enter_context(tc.tile_pool(name="consts", bufs=1))

        whh_sb = [[consts.tile([128, G], bf16, name=f"whh{c}{k}",
                               tag=f"whh{c}{k}") for k in range(2)]
                  for c in range(2)]
        for c in range(2):
            for k in range(2):
                nc.sync.dma_start(out=whh_sb[c][k],
                                  in_=whh[c, k * 128:(k + 1) * 128, :])

        scat_sb = consts.tile([2 * R, 40], f32r, tag="scat_sb")
        nc.sync.dma_start(out=scat_sb, in_=scat[:, :])
        id8b = consts.tile([R, R], bf16, tag="id8b")
        make_identity(nc, id8b)

        # ---------------- phase 1: gx = x @ WihT + bias ----------------
        with ExitStack() as p1:
          if not skip1:
            p1c = p1.enter_context(tc.tile_pool(name="p1c", bufs=1))
            xp = p1.enter_context(tc.tile_pool(name="xp", bufs=4))
            gp = p1.enter_context(tc.tile_pool(name="gp", bufs=2, space="PSUM"))
            gs = p1.enter_context(tc.tile_pool(name="gs", bufs=4))

            wih_sb = [p1c.tile([I, G], f32r, name=f"wih{c}", tag=f"wih{c}")
                      for c in range(2)]
            bia_sb = [p1c.tile([128, G], f32, name=f"bia{c}", tag=f"bia{c}")
                      for c in range(2)]
            for c in range(2):
                nc.sync.dma_start(out=wih_sb[c], in_=wih[c])
                bsrc = bia[c:c + 1, :]
                nc.sync.dma_start(
                    out=bia_sb[c],
                    in_=bass.AP(tensor=bsrc.tensor, offset=bsrc.offset,
                                ap=[[0, 128]] + list(bsrc.ap[-1:])))

            for c in range(2):
                for s in range(2):
                    src = xsrc(c, s)
                    for b in range(BL):
                        row = s * BL + b
                        for t in range(TC):
                            xt = xp.tile([I, 128], f32r, tag="xt")
                            nc.sync.dma_start(
                                out=xt, in_=src[b, :, t * 128:(t + 1) * 128])
                            ps = gp.tile([128, G], f32, tag="ps")
                            for n in range(2):
                                sl = ds(n * 512, 512)
                                nc.tensor.matmul(ps[:, sl], xt,
                                                 wih_sb[c][:, sl],
                                                 start=True, stop=True)
                            gt = gs.tile([128, G], f32r, tag="gt")
                            nc.vector.tensor_add(gt, ps, bia_sb[c])
                            tsl = ds(t * 128, 128)
                            nc.sync.dma_start(
                                out=gxst[c][tsl, row, :], in_=gt[:, 0:512])
                            nc.sync.dma_start(
                                out=gxst[c][tsl, R + row, :],
                                in_=gt[:, 512:1024])

        # ---------------- phase 2: the recurrence ----------------
        with ExitStack() as p2:
          if not skip2:
            st8 = p2.enter_context(tc.tile_pool(name="st8", bufs=1))
            gxp = p2.enter_context(tc.tile_pool(name="gxp", bufs=2))
            spl = p2.enter_context(tc.tile_pool(name="spl", bufs=2))
            tmp = p2.enter_context(tc.tile_pool(name="tmp", bufs=2))
            stg = p2.enter_context(tc.tile_pool(name="stg", bufs=2))
            psA = p2.enter_context(tc.tile_pool(name="psA", bufs=3, space="PSUM"))
            psB = p2.enter_context(tc.tile_pool(name="psB", bufs=3, space="PSUM"))
            psT = p2.enter_context(tc.tile_pool(name="psT", bufs=1, space="PSUM"))

            # persistent state (ping-pong on step parity), split per chain
            # so the two chains' dependency cycles stay independent.
            # hTc[c][p]: [128, 16] cols hb*8..+8 = h[c]^T chunk hb
            # c state lives at base partition 32 (rows 32:40) to match the
            # [f|o] gate zone
            hTc = [[st8.tile([128, 2 * R], bf16, name=f"hT{c}{p}",
                             tag=f"hT{c}{p}") for p in range(2)]
                   for c in range(2)]
            cst = [[st8.tile([40, H], f32, name=f"c{c}{p}", tag=f"c{c}{p}")
                    for p in range(2)] for c in range(2)]
            for p in range(2):
                for c in range(2):
                    nc.vector.memset(hTc[c][p], 0.0)
                    nc.vector.memset(cst[c][p], 0.0)

            stt = nc.vector.scalar_tensor_tensor

            # persistent gx head: steps 0..3 of the NEXT iteration are
            # prefetched mid-iteration, before the loop barrier
            gxh = [st8.tile([2 * R, 4, 512], f32r, name=f"gxh{c}",
                            tag=f"gxh{c}") for c in range(2)]
            for c in range(2):
                nc.sync.dma_start(
                    out=gxh[c],
                    in_=gxst[c][ds(0, 4), :, :].rearrange("u p g -> p u g"))

            def hT_sl(p, c, hb):
                return hTc[c][p][:, hb * R:(hb + 1) * R]

            with tc.For_i(0, T, U, staggered_reset=True) as iv:
                gxch = [gxp.tile([2 * R, U - 4, 512], f32r, name=f"gx{c}",
                                 tag=f"gx{c}") for c in range(2)]
                for c in range(2):
                    # tail: steps 4..U-1 of this iteration (the head tiles
                    # already hold steps 0..3, loaded last iteration)
                    nc.sync.dma_start(
                        out=gxch[c],
                        in_=gxst[c][ds(iv + 4, U - 4), :, :].rearrange(
                            "u p g -> p u g"))
                st = [stg.tile([R, U, H], bf16, name=f"st{c}", tag=f"st{c}")
                      for c in range(2)]

                def scatter(c, kk):
                    pool = psA if c == 0 else psB
                    g_ = pool.tile([40, 512], f32, name=f"G{c}", tag=f"G{c}")
                    src_ = (gxh[c][:, kk, :] if kk < 4
                            else gxch[c][:, kk - 4, :])
                    nc.tensor.matmul(g_, scat_sb, src_,
                                     start=True, stop=True)
                    return g_

                Gq = []
                for k in range(U):
                    pp = k % 2
                    if k == 0:
                        Gq = [[scatter(c, 0) for c in range(2)],
                              [scatter(c, 1) for c in range(2)]]
                    Gcur = Gq.pop(0)
                    # prefetch gx two steps ahead into fresh psum banks:
                    # the PE runs these while the gate matmuls below still
                    # wait on the recurrent state
                    if k + 2 < U:
                        Gq.append([scatter(c, k + 2) for c in range(2)])
                    # gate matmuls accumulate onto gx (h-chunk outer so
                    # the first two only wait on hT chunk 0)
                    for c in range(2):
                        for kc in range(2):
                            for s in range(2):
                                nc.tensor.matmul(
                                    Gcur[c][s * 32:s * 32 + R, :],
                                    hT_sl(pp, c, kc),
                                    whh_sb[c][kc][:, s * 512:(s + 1) * 512],
                                    start=False, stop=True,
                                    skip_group_check=True)
                    # sigmoids (gate cols permuted on host to [i g | f o]:
                    # zone0 rows 0:8 = i|g, zone32 rows 32:40 = f|o)
                    S, slc = [], {}
                    for c in range(2):
                        S_ = spl.tile([40, 512], bf16, name=f"S{c}",
                                      tag=f"S{c}")
                        nc.scalar.activation(S_, Gcur[c], SIG)
                        S.append(S_)
                        slc[c] = (S_[0:R, 0:256], S_[32:32 + R, 0:256],
                                  S_[0:R, 256:512], S_[32:32 + R, 256:512])
                    # c update: GPSIMD takes v = Sf*c (off the critical
                    # cycle); DVE runs each chain's dependent tail
                    # [p, u, cn, h] consecutively so chain A's tail is not
                    # queued behind chain B's ops
                    for c in range(2):
                        Sf = slc[c][1]
                        v_ = tmp.tile([R, H], f32, name=f"v{c}", tag=f"v{c}")
                        nc.gpsimd.tensor_mul(v_, Sf, cst[c][pp][32:40, :])
                        slc[c] = slc[c] + (v_,)
                    for c in range(2):
                        Si, Sf, Sg, So, v_ = slc[c]
                        p_ = tmp.tile([R, H], bf16, name=f"p{c}", tag=f"p{c}")
                        nc.vector.tensor_mul(p_, Si, Sg)
                        u_ = tmp.tile([R, H], bf16, name=f"u{c}", tag=f"u{c}")
                        stt(out=u_, in0=p_, scalar=2.0, in1=Si,
                            op0=MULT, op1=SUB)
                        cn = cst[c][1 - pp][32:40, :]
                        nc.vector.tensor_add(cn, u_, v_)
                        tc_ = tmp.tile([40, H], bf16, name=f"tc{c}",
                                       tag=f"tc{c}")
                        nc.scalar.activation(tc_[32:40, :], cn, TANH)
                        # h = sig(o) * tanh(c) -> store buffer (bf16)
                        nc.vector.tensor_mul(st[c][:, k, :], So,
                                             tc_[32:40, :])
                        ptt = psT.tile([128, 2 * R], bf16, name=f"ptt{c}",
                                       tag=f"ptt{c}")
                        # per-chunk transpose+copy: the next step's hb=0
                        # matmuls only wait for chunk 0
                        nc.tensor.transpose(ptt[:, 0:R],
                                            st[c][:, k, 0:128], id8b)
                        nc.scalar.copy(hTc[c][1 - pp][:, 0:R], ptt[:, 0:R])
                        nc.tensor.transpose(ptt[:, R:2 * R],
                                            st[c][:, k, 128:256], id8b)
                        nc.vector.tensor_copy(hTc[c][1 - pp][:, R:2 * R],
                                              ptt[:, R:2 * R])
                    if k == 7:
                        for c in range(2):
                            nc.sync.dma_start(
                                out=gxh[c],
                                in_=gxst[c][ds(iv + U, 4), :, :].rearrange(
                                    "u p g -> p u g"))
                            nc.gpsimd.dma_start(
                                out=hst[c][ds(iv, 8), :, :].rearrange(
                                    "u p h -> p u h"),
                                in_=st[c][:, 0:8, :])
                for c in range(2):
                    nc.gpsimd.dma_start(
                        out=hst[c][ds(iv + 8, U - 8), :, :].rearrange(
                            "u p h -> p u h"),
                        in_=st[c][:, 8:U, :])

        # ------- phase 3: combine, transpose to (b, h, t), interleave -----
        with ExitStack() as p3:
          if not skip3:
            p3c = p3.enter_context(tc.tile_pool(name="p3c", bufs=1))
            lp = p3.enter_context(tc.tile_pool(name="lp", bufs=4))
            cmb = p3.enter_context(tc.tile_pool(name="cmb", bufs=4))
            tp = p3.enter_context(tc.tile_pool(name="tp", bufs=4, space="PSUM"))
            op = p3.enter_context(tc.tile_pool(name="op", bufs=4))

            id128 = p3c.tile([128, 128], f32, tag="id128")
            make_identity(nc, id128)

            for b in range(BL):
                for t in range(TC):
                    tsl = ds(t * 128, 128)
                    a0 = lp.tile([128, H], bf16, tag="a0")
                    b0 = lp.tile([128, H], bf16, tag="b0")
                    a1 = lp.tile([128, H], bf16, tag="a1")
                    b1 = lp.tile([128, H], bf16, tag="b1")
                    nc.sync.dma_start(out=a0, in_=hst[0][tsl, b, :])
                    nc.sync.dma_start(out=b0, in_=hst[1][tsl, b, :])
                    nc.sync.dma_start(out=a1, in_=hst[0][tsl, BL + b, :])
                    nc.sync.dma_start(out=b1, in_=hst[1][tsl, BL + b, :])
                    lr = cmb.tile([128, H], f32, tag="lr")
                    nc.vector.tensor_sub(lr, a0, b0)
                    li = cmb.tile([128, H], f32, tag="li")
                    nc.gpsimd.tensor_add(li, a1, b1)
                    for hb in range(2):
                        hsl = ds(hb * 128, 128)
                        ptr = tp.tile([128, 128], f32, tag="ptr")
                        nc.tensor.transpose(ptr, lr[:, hsl], id128)
                        pti = tp.tile([128, 128], f32, tag="pti")
                        nc.tensor.transpose(pti, li[:, hsl], id128)
                        ot = op.tile([128, 256], f32, tag="ot")
                        otv = ot.rearrange("p (t two) -> p t two", two=2)
                        nc.vector.tensor_copy(otv[:, :, 0], ptr)
                        nc.vector.tensor_copy(otv[:, :, 1], pti)
                        nc.sync.dma_start(
                            out=out[b, hsl, ds(2 * t * 128, 256)], in_=ot)

    nc.compile()
    return nc


_CACHE = {}
LAST_RES = None


def get_program(T):
    if T not in _CACHE:
        _CACHE[T] = build_program(T)
    return _CACHE[T]


def _pack_weights(Wih, Whh, bih, bhh):
    Wih = np.array(Wih, dtype=np.float32, copy=True)
    Whh = np.array(Whh, dtype=np.float32, copy=True)
    b = (np.asarray(bih) + np.asarray(bhh)).astype(np.float32)
    # pre-scale g gate (rows 2H:3H) by 2 so sigmoid(2g) gives tanh via 2s-1
    Wih[2 * H:3 * H] *= 2.0
    Whh[2 * H:3 * H] *= 2.0
    b[2 * H:3 * H] *= 2.0
    # permute gate blocks (i, f, g, o) -> (i, g, f, o) so the kernel's
    # zone0 = [i|g], zone32 = [f|o]
    perm = np.r_[0:H, 2 * H:3 * H, H:2 * H, 3 * H:4 * H]
    Wih = Wih[perm]
    Whh = Whh[perm]
    b = b[perm]
    return np.ascontiguousarray(Wih.T), np.ascontiguousarray(Whh.T), b


def kernel(x_real, x_imag, Wih_r, Whh_r, bih_r, bhh_r,
           Wih_i, Whh_i, bih_i, bhh_i):
    x_real = np.asarray(x_real, dtype=np.float32)
    x_imag = np.asarray(x_imag, dtype=np.float32)
    T = x_real.shape[2]
    nc = get_program(T)

    wihT_r, whhT_r, b_r = _pack_weights(Wih_r, Whh_r, bih_r, bhh_r)
    wihT_i, whhT_i, b_i = _pack_weights(Wih_i, Whh_i, bih_i, bhh_i)
    wih_p = np.ascontiguousarray(np.stack([wihT_r, wihT_i]))
    import ml_dtypes
    whh_p = np.ascontiguousarray(
        np.stack([whhT_r, whhT_i]).astype(ml_dtypes.bfloat16))
    bia_p = np.ascontiguousarray(np.stack([b_r, b_i]))
    scat_p = np.zeros((2 * R, 40), dtype=np.float32)
    for j in range(R):
        scat_p[j, j] = 1.0
        scat_p[R + j, 32 + j] = 1.0

    in_maps = []
    for c in range(NCORES):
        sl = slice(c * BL, (c + 1) * BL)
        in_maps.append({
            "xr": np.ascontiguousarray(x_real[sl]),
            "xi": np.ascontiguousarray(x_imag[sl]),
            "wih": wih_p, "whh": whh_p, "bias": bia_p,
            "scat": scat_p,
        })
    import os
    trace = os.environ.get("K_TRACE") == "1"
    res = run_bass_kernel_spmd(nc, in_maps, list(range(NCORES)), trace=trace)
    global LAST_RES
    LAST_RES = res
    parts = []
    for c in range(NCORES):
        o = np.ascontiguousarray(res.results[c]["out"])  # [BL, H, 2T] f32
        parts.append(o.view(np.complex64))               # [BL, H, T]
    return np.concatenate(parts, axis=0)
